# revision 17
# baseline (speedup 1.0000x reference)
"""AlignmentContrastiveLoss Trainium2 kernel.

Math (matching the reference):
  im = im_set[:, 1:, :]        -> [128, 64, 1024]  (rows bi = (b, i))
  s  = s_seq[:, 1:-2, :]       -> [128, 64, 1024]  (rows tj = (t, j))
  align[b,t,i,j] = im[b,i,:] . s[t,j,:]   (masked entries forced to 0)
  aggr[b,t] = sum_j max_i align
  loss = hinge-contrastive reduction of aggr [128,128]  (tiny, done on host)

Device strategy (8 NeuronCores, SPMD):
  - Shard sentences: core c owns 16 sentences (1024 tj rows), streams all
    8192 image rows.
  - Both matmul operands need D on partitions, so natural-layout tiles are
    transposed on the PE (fp32 DMA transpose doesn't exist on trn2).
  - Matmuls run in fp16 (PE upconverts to fp22, fp32 PSUM accumulate):
    full PE rate at N=512, ~1e-5 relative loss accuracy. fp16 also halves
    SBUF pressure and uses the standard LDWEIGHTS+MATMUL lowering (the
    fp32r self-loading path only fits one semaphore wait per instruction
    and fails walrus codegen under Tile's sync placement).
  - Padded image rows are zeroed on device (ACT copy scaled by a 0/1
    per-partition mask) before the transpose, so max_i sees exactly the
    zeros the reference's masking produces.
  - i-max is a segmented (64-wide) free-dim reduce of each PSUM slab;
    j-sum is a matmul with a [128,2] block-indicator against the maxes,
    after scaling by the 0/1 word mask.
  - Each core emits aggr for its 16 sentences x 128 images; host assembles
    aggr [128,128] and computes the scalar hinge loss.
"""

import numpy as np

import concourse.bass as bass
import concourse.mybir as mybir
import concourse.tile as tile
from concourse.bass_utils import run_bass_kernel_spmd

F32 = mybir.dt.float32
F16 = mybir.dt.float16

MARGIN = 0.2
B = 128
LI = 64          # image regions after slicing
LS = 64          # words after slicing
D = 1024
NCORES = 8
T_PER_CORE = B // NCORES            # 16 sentences per core
TJ = T_PER_CORE * LS                # 1024 local s rows
BI = B * LI                         # 8192 image rows (streamed by every core)
KT = D // 128                       # 8 contraction tiles
NTAU = TJ // 128                    # 8 tj tiles of 128
G = 8                               # image-row groups
GROUP_ROWS = BI // G                # 1024 rows per group
NAT_PER_GROUP = GROUP_ROWS // 128   # 8 natural [128, D] tiles per group


def _build_nc(prune=True, detect_races=True):
    from contextlib import ExitStack

    nc = bass.Bass(detect_race_conditions=detect_races)
    s_own = nc.dram_tensor("s_own", [TJ, D], F32, kind="ExternalInput")
    im_all = nc.dram_tensor("im_all", [BI, D], F32, kind="ExternalInput")
    # immask[p, n] = 0/1 validity of image row n*128+p
    immask = nc.dram_tensor("immask", [128, BI // 128], F32, kind="ExternalInput")
    # smask[p, tau] = 0/1 validity of local s row tau*128+p
    smask = nc.dram_tensor("smask", [128, NTAU], F32, kind="ExternalInput")
    # ones_e[p, h] = 1 if p // 64 == h (sums 64-partition halves via matmul)
    ones_e = nc.dram_tensor("ones_e", [128, 2], F16, kind="ExternalInput")
    ident_in = nc.dram_tensor("ident_in", [128, 128], F16, kind="ExternalInput")
    aggr_out = nc.dram_tensor("aggr_out", [2, TJ], F32, kind="ExternalOutput")

    with tile.TileContext(nc) as tc, ExitStack() as ctx:
        consts = ctx.enter_context(tc.tile_pool(name="consts", bufs=1))
        natp = ctx.enter_context(tc.tile_pool(name="natp", bufs=6))
        nat16p = ctx.enter_context(tc.tile_pool(name="nat16p", bufs=8))
        imtp = ctx.enter_context(tc.tile_pool(name="imtp", bufs=2))
        mp = ctx.enter_context(tc.tile_pool(name="mp", bufs=1))
        outp = ctx.enter_context(tc.tile_pool(name="outp", bufs=1))
        pst = ctx.enter_context(tc.tile_pool(name="pst", bufs=2, space="PSUM"))
        psm = ctx.enter_context(tc.tile_pool(name="psm", bufs=4, space="PSUM"))
        psf = ctx.enter_context(tc.tile_pool(name="psf", bufs=1, space="PSUM"))

        ident = consts.tile([128, 128], F16)
        nc.sync.dma_start(ident[:], ident_in[:])

        immask_sb = consts.tile([128, BI // 128], F32)
        nc.sync.dma_start(immask_sb[:], immask[:])
        smask_sb = consts.tile([128, NTAU], F32)
        nc.sync.dma_start(smask_sb[:], smask[:])
        e_sb = consts.tile([128, 2], F16)
        nc.sync.dma_start(e_sb[:], ones_e[:])

        # sT[:, tau, k, :] = s_own[tau*128:(tau+1)*128, k*128:(k+1)*128].T
        sT = consts.tile([128, NTAU, KT, 128], F16)
        # m_all[p, tau*128 + b] = max_i of masked align for s row (tau, p)
        m_all = mp.tile([128, TJ], F16)

        # DVE touches the mask tiles once up front so later DVE ops never
        # need a second (DMA) wait for them; each TPB instruction can only
        # encode one semaphore wait.
        dummy_sb = consts.tile([128, 3], F32)
        nc.vector.tensor_copy(dummy_sb[:, 0:1], immask_sb[:, 0:1])
        nc.vector.tensor_copy(dummy_sb[:, 1:2], smask_sb[:, 0:1])
        nc.vector.tensor_copy(dummy_sb[:, 2:3], e_sb[:, 0:1])

        def transpose_nat(nat16, dst, dst_off):
            """8 PE transposes of one [128, D] fp16 tile into one PSUM bank,
            then a single DVE copy into dst[:, :, dst_off:dst_off+128]."""
            pt = pst.tile([128, KT, 128], F16)
            for k in range(KT):
                nc.tensor.transpose(
                    pt[:, k], nat16[:, k * 128:(k + 1) * 128], ident[:]
                )
            nc.vector.tensor_copy(dst[:, :, dst_off:dst_off + 128], pt[:])

        # Phase A: own sentences -> fp16 -> transposed [d, row] layout.
        for tau in range(NTAU):
            nat = natp.tile([128, D], F32, tag="nat")
            nc.sync.dma_start(nat[:], s_own[tau * 128:(tau + 1) * 128, :])
            nat16 = nat16p.tile([128, D], F16, tag="nat16")
            nc.vector.tensor_copy(nat16[:], nat[:])
            transpose_nat(nat16, sT[:, tau], 0)

        # Phase B: stream image groups; transpose, matmul, segment-max.
        for g in range(G):
            imt = imtp.tile([128, KT, GROUP_ROWS], F16)
            for n in range(NAT_PER_GROUP):
                nidx = g * NAT_PER_GROUP + n
                nat = natp.tile([128, D], F32, tag="nat")
                nc.sync.dma_start(nat[:], im_all[nidx * 128:(nidx + 1) * 128, :])
                nat16 = nat16p.tile([128, D], F16, tag="nat16")
                # cast to fp16 and zero padded image rows in one DVE op;
                # running this on DVE (not ACT) lets the engine-observed
                # clock cover the nat16 recycle dependency.
                nc.vector.tensor_scalar_mul(
                    nat16[:], nat[:], immask_sb[:, nidx:nidx + 1]
                )
                transpose_nat(nat16, imt, n * 128)

            for tau in range(NTAU):
                for c in range(2):
                    pm = psm.tile([128, 512], F32)
                    for k in range(KT):
                        nc.tensor.matmul(
                            pm[:],
                            sT[:, tau, k, :],
                            imt[:, k, c * 512:(c + 1) * 512],
                            start=(k == 0),
                            stop=(k == KT - 1),
                        )
                    base = tau * 128 + g * 16 + c * 8
                    nc.vector.reduce_max(
                        m_all[:, base:base + 8],
                        pm[:].rearrange("p (i j) -> p i j", j=LI),
                        axis=mybir.AxisListType.X,
                    )

        # Phase C: zero padded words, then sum over words via ones-matmul.
        for tau in range(NTAU):
            nc.vector.tensor_scalar_mul(
                m_all[:, tau * 128:(tau + 1) * 128],
                m_all[:, tau * 128:(tau + 1) * 128],
                smask_sb[:, tau:tau + 1],
            )
        pf = psf.tile([2, 2, 512], F32)
        for h in range(2):
            nc.tensor.matmul(
                pf[:, h],
                e_sb[:],
                m_all[:, h * 512:(h + 1) * 512],
                start=True,
                stop=True,
            )
        out_sb = outp.tile([2, TJ], F32)
        nc.vector.tensor_copy(out_sb[:], pf[:].rearrange("p a b -> p (a b)"))
        nc.sync.dma_start(aggr_out[:], out_sb[:])

    if prune:
        _prune_redundant_waits(nc)
    return nc


def _prune_redundant_waits(nc):
    """Drop semaphore waits that are provably redundant on the final schedule.

    Walrus's per-instruction ISA structs encode very few sync waits (one for
    PE Matmult / HWDGE DMA), and Tile's wait placement leaves redundant ones:
    (a) waits on the instruction's own processor semaphore (PE matmuls
    complete in program order; a HWDGE queue executes its descriptors FIFO),
    and (b) waits whose target completion is already in the causal past of
    another wait kept on the same instruction. Both classes are dropped here
    using a conservative happens-before computed from the untouched program.

    "Processor" is the engine, except DMACopy where it is the HW queue
    (identified by its update semaphore). Ldweights can be pulled ahead of
    in-flight matmuls by the PE, so it neither extends nor inherits the
    same-proc completion chain.
    """
    insts = []
    for f in nc.m.functions:
        for bb in f.blocks:
            insts.extend(bb.instructions)

    def proc_of(i, idx):
        if i.opcode == "DMACopy":
            ups = i.sync_info.on_update
            qs = [u.ant_name for u in ups if "DMA" in u.ant_name]
            if len(qs) == 1:
                return qs[0]
            return f"__solo_{idx}"
        return f"__eng_{i.engine}"

    # completion clocks: clock[i] = {sem: min guaranteed value when i completes}
    sem_events = {}   # sem -> list of (cumval, inst_idx) in inc order
    sem_cum = {}
    clocks = [None] * len(insts)
    last_in_proc = {}

    def join(a, b):
        for k, v in b.items():
            if a.get(k, -1) < v:
                a[k] = v
        return a

    def producer_clock(sem, val):
        evs = sem_events.get(sem)
        if not evs:
            return None
        # first event reaching val
        import bisect
        pos = bisect.bisect_left(evs, (val, -1))
        if pos == len(evs):
            return None
        return clocks[evs[pos][1]]

    class _EmptySI:
        on_wait = ()
        on_update = ()

    for idx, i in enumerate(insts):
        si = i.sync_info or _EmptySI
        c = {}
        if i.opcode != "Ldweights":
            p = proc_of(i, idx)
            prev = last_in_proc.get(p)
            if prev is not None:
                join(c, clocks[prev])
            last_in_proc[p] = idx
        for w in si.on_wait:
            pc = producer_clock(w.ant_name, w.wait_value)
            if pc is not None:
                join(c, pc)
            if c.get(w.ant_name, -1) < w.wait_value:
                c[w.ant_name] = w.wait_value
        for u in si.on_update:
            sem = u.ant_name
            cum = sem_cum.get(sem, 0) + u.update_value
            sem_cum[sem] = cum
            sem_events.setdefault(sem, []).append((cum, idx))
            if c.get(sem, -1) < cum:
                c[sem] = cum
        clocks[idx] = c

    # pruning pass, walking issue order per processor:
    #   (a) waits on the instruction's own processor semaphore (in-order
    #       completion within a processor),
    #   (b) waits transitively covered by another kept wait's causal past,
    #   (c) waits at-or-below what an earlier instruction on the same
    #       issue processor already waited for (semaphores are monotone).
    PRUNABLE = {
        "Matmult", "Ldweights", "DMACopy", "Activation", "TensorCopy",
        "TensorReduce", "TensorScalarPtr", "TensorTensor", "Memset",
        "Drain",
    }
    stripped = 0
    observed = {}  # issue proc -> {sem: max value already waited for}

    for idx, i in enumerate(insts):
        si = i.sync_info
        if si is None:
            continue
        p = proc_of(i, idx)
        obs = observed.setdefault(p, {})
        waits = list(si.on_wait)
        if i.opcode in PRUNABLE and waits:
            eng = str(i.engine).split(".")[-1]
            kept = []
            for w in waits:
                sem_eng = w.ant_name.rsplit("_", 1)[0]
                # rule (a): same-engine completion is in program order, so a
                # wait on the engine's own semaphore is vacuous. NOT applied
                # to DMA self-queue waits: a queue's sem increments are only
                # ordered if the previous transfer provably completed, which
                # is rule (b)'s job.
                if i.opcode != "DMACopy" and sem_eng == eng:
                    continue
                if obs.get(w.ant_name, -1) >= w.wait_value:
                    continue           # rule (c): already observed
                kept.append(w)
            # rule (b): transitive cover by other kept waits (or observed)
            changed = True
            while changed and len(kept) > 1:
                changed = False
                for w in list(kept):
                    cover = {}
                    for x in kept:
                        if x is w:
                            continue
                        pc = producer_clock(x.ant_name, x.wait_value)
                        if pc is not None:
                            join(cover, pc)
                    if cover.get(w.ant_name, -1) >= w.wait_value:
                        kept.remove(w)
                        changed = True
            if len(kept) != len(waits):
                si.on_wait = kept
                stripped += 1
            waits = kept
        for w in waits:
            if obs.get(w.ant_name, -1) < w.wait_value:
                obs[w.ant_name] = w.wait_value
    return stripped


_NC_CACHE = None


def _get_nc():
    global _NC_CACHE
    if _NC_CACHE is None:
        _NC_CACHE = _build_nc()
    return _NC_CACHE


def _prepare_in_maps(im_set, s_seq, im_len, s_len):
    im_set = np.asarray(im_set, dtype=np.float32)
    s_seq = np.asarray(s_seq, dtype=np.float32)
    im_l = np.asarray(im_len).astype(np.int64) - 1
    s_l = np.asarray(s_len).astype(np.int64) - 3

    im = np.ascontiguousarray(im_set[:, 1:, :]).reshape(BI, D)
    immask_full = (np.arange(LI)[None, :] < im_l[:, None]).astype(np.float32)
    immask_dev = np.ascontiguousarray(
        immask_full.reshape(BI // 128, 128).T
    )
    smask_full = (np.arange(LS)[None, :] < s_l[:, None]).astype(np.float32)
    smask_flat = smask_full.reshape(B * LS)

    ones_e = np.zeros((128, 2), np.float16)
    ones_e[:64, 0] = 1.0
    ones_e[64:, 1] = 1.0
    ident = np.eye(128, dtype=np.float16)

    in_maps = []
    for c in range(NCORES):
        s_own = np.ascontiguousarray(
            s_seq[c * T_PER_CORE:(c + 1) * T_PER_CORE, 1:1 + LS, :]
        ).reshape(TJ, D)
        smask_own = np.ascontiguousarray(
            smask_flat[c * TJ:(c + 1) * TJ].reshape(NTAU, 128).T
        )
        in_maps.append(
            {
                "s_own": s_own,
                "im_all": im,
                "immask": immask_dev,
                "smask": smask_own,
                "ones_e": ones_e,
                "ident_in": ident,
            }
        )
    return in_maps


def _loss_from_cores(core_outs):
    aggr = np.zeros((B, B), np.float64)
    for c in range(NCORES):
        o = np.asarray(core_outs[c], dtype=np.float64).reshape(2, NTAU, 128)
        for tau in range(NTAU):
            for h in range(2):
                aggr[:, c * T_PER_CORE + 2 * tau + h] = o[h, tau, :]
    diag = np.diag(aggr)
    cost_s = MARGIN + aggr - diag[:, None]
    cost_im = MARGIN + aggr - diag[None, :]
    np.fill_diagonal(cost_s, 0.0)
    np.fill_diagonal(cost_im, 0.0)
    cost_s = np.maximum(cost_s, 0.0)
    cost_im = np.maximum(cost_im, 0.0)
    loss = cost_s.max(axis=1).sum() + cost_im.max(axis=0).sum()
    return np.array(loss, dtype=np.float32)


def _run(im_set, s_seq, im_len, s_len, **spmd_kwargs):
    nc = _get_nc()
    in_maps = _prepare_in_maps(im_set, s_seq, im_len, s_len)
    res = run_bass_kernel_spmd(
        nc, in_maps, core_ids=list(range(NCORES)), **spmd_kwargs
    )
    loss = _loss_from_cores([r["aggr_out"] for r in res.results])
    return loss, res


def kernel(im_set, s_seq, im_len, s_len):
    loss, _ = _run(im_set, s_seq, im_len, s_len)
    return loss


def _install_ntff_hook_shim():
    """This image's antenv lacks axon_hooks; recreate it from trn_boot's
    ctypes path so run_bass_kernel_spmd(trace=True) can capture NTFFs."""
    import sys
    import types

    if "antenv.axon_hooks" in sys.modules:
        return
    from trn_agent_boot.trn_boot import _ntff_profile_via_ctypes

    hook = _ntff_profile_via_ctypes("/opt/axon/libaxon_pjrt.so")
    mod = types.ModuleType("antenv.axon_hooks")
    mod._hook = hook
    mod.get_axon_ntff_profile_hook = lambda: mod._hook
    mod.set_axon_ntff_profile_hook = lambda h: setattr(mod, "_hook", h)
    sys.modules["antenv.axon_hooks"] = mod
    import antenv

    antenv.axon_hooks = mod


def kernel_traced(im_set, s_seq, im_len, s_len, **kwargs):
    """Returns (loss, BassKernelResults-with-exec_time_ns)."""
    _install_ntff_hook_shim()
    loss, res = _run(im_set, s_seq, im_len, s_len, trace=True, **kwargs)
    return loss, res


# revision 36
# speedup vs baseline: 1.2673x; 1.2673x over previous
"""AlignmentContrastiveLoss Trainium2 kernel.

Math (matching the reference):
  im = im_set[:, 1:, :]        -> [128, 64, 1024]  (rows bi = (b, i))
  s  = s_seq[:, 1:-2, :]       -> [128, 64, 1024]  (rows tj = (t, j))
  align[b,t,i,j] = im[b,i,:] . s[t,j,:]   (masked entries forced to 0)
  aggr[b,t] = sum_j max_i align
  loss = hinge-contrastive reduction of aggr [128,128]  (tiny, done on host)

Device strategy (8 NeuronCores, SPMD):
  - Shard sentences: core c owns 16 sentences (1024 tj rows), streams all
    8192 image rows.
  - Both matmul operands need D on partitions, so natural-layout tiles are
    transposed on the PE (fp32 DMA transpose doesn't exist on trn2).
  - Matmuls run in fp16 (PE upconverts to fp22, fp32 PSUM accumulate):
    full PE rate at N=512, ~1e-5 relative loss accuracy. fp16 also halves
    SBUF pressure and uses the standard LDWEIGHTS+MATMUL lowering (the
    fp32r self-loading path only fits one semaphore wait per instruction
    and fails walrus codegen under Tile's sync placement).
  - Padded image rows and padded s word rows are zeroed on device (DVE
    tensor_scalar with a 0/1 per-partition mask) before the transposes,
    so masked align entries are exactly 0, as in the reference.
  - i-max is a segmented (64-wide) free-dim reduce of each PSUM slab;
    j-sum is a matmul with a [128,2] block-indicator against the maxes.
  - A post-Tile pass prunes/migrates redundant semaphore waits: the TPB
    ISA encodes ONE wait per instruction and Tile's placement exceeds
    that; see _prune_redundant_waits.
  - Each core emits aggr for its 16 sentences x 128 images; host assembles
    aggr [128,128] and computes the scalar hinge loss.
"""

import numpy as np

import concourse.bass as bass
import concourse.mybir as mybir
import concourse.tile as tile
from concourse.bass_utils import run_bass_kernel_spmd

F32 = mybir.dt.float32
F16 = mybir.dt.float16

MARGIN = 0.2
B = 128
LI = 64          # image regions after slicing
LS = 64          # words after slicing
D = 1024
NCORES = 8
T_PER_CORE = B // NCORES            # 16 sentences per core
TJ = T_PER_CORE * LS                # 1024 local s rows
BI = B * LI                         # 8192 image rows (streamed by every core)
KT = D // 128                       # 8 contraction tiles
NTAU = TJ // 128                    # 8 tj tiles of 128
G = 4                               # image-row groups
GROUP_ROWS = BI // G                # 2048 rows per group
CCH = GROUP_ROWS // 512             # 512-wide psum chunks per group
NAT_PER_GROUP = GROUP_ROWS // 128   # 8 natural [128, D] tiles per group


def _build_nc(prune=True, detect_races=True):
    from contextlib import ExitStack

    nc = bass.Bass(detect_race_conditions=detect_races)
    s_own = nc.dram_tensor("s_own", [TJ, D], F16, kind="ExternalInput")
    im_all = nc.dram_tensor("im_all", [BI, D], F16, kind="ExternalInput")
    # immask[p, n] = 0/1 validity of image row n*128+p
    immask = nc.dram_tensor("immask", [128, BI // 128], F32, kind="ExternalInput")
    # smask[p, tau] = 0/1 validity of local s row tau*128+p
    smask = nc.dram_tensor("smask", [128, NTAU], F32, kind="ExternalInput")
    # ones_e[p, h] = 1 if p // 64 == h (sums 64-partition halves via matmul)
    ones_e = nc.dram_tensor("ones_e", [128, 2], F16, kind="ExternalInput")
    ident_in = nc.dram_tensor("ident_in", [128, 128], F16, kind="ExternalInput")
    aggr_out = nc.dram_tensor("aggr_out", [2, TJ], F32, kind="ExternalOutput")

    with tile.TileContext(nc) as tc, ExitStack() as ctx:
        consts = ctx.enter_context(tc.tile_pool(name="consts", bufs=1))
        natp = ctx.enter_context(tc.tile_pool(name="natp", bufs=8))
        nat16p = ctx.enter_context(tc.tile_pool(name="nat16p", bufs=36))
        imtp = ctx.enter_context(tc.tile_pool(name="imtp", bufs=2))
        mp = ctx.enter_context(tc.tile_pool(name="mp", bufs=1))
        outp = ctx.enter_context(tc.tile_pool(name="outp", bufs=1))
        pst = ctx.enter_context(tc.tile_pool(name="pst", bufs=2, space="PSUM"))
        psm = ctx.enter_context(tc.tile_pool(name="psm", bufs=5, space="PSUM"))

        ident = consts.tile([128, 128], F16)
        nc.sync.dma_start(ident[:], ident_in[:])

        immask_sb = consts.tile([128, BI // 128], F32)
        nc.sync.dma_start(immask_sb[:], immask[:])
        smask_sb = consts.tile([128, NTAU], F32)
        nc.sync.dma_start(smask_sb[:], smask[:])
        e_sb = consts.tile([128, 2], F16)
        nc.sync.dma_start(e_sb[:], ones_e[:])

        # sT[:, tau, k, :] = s_own[tau*128:(tau+1)*128, k*128:(k+1)*128].T
        sT = consts.tile([128, NTAU, KT, 128], F16)
        # m_all[p, tau*128 + b] = max_i of masked align for s row (tau, p)
        m_all = mp.tile([128, TJ], F16)

        # DVE touches the mask tiles once up front so later DVE ops never
        # need a second (DMA) wait for them; each TPB instruction can only
        # encode one semaphore wait.
        tch = consts.tile([128, 1], F16)
        dummy_sb = consts.tile([128, 3], F32)
        nc.vector.tensor_copy(dummy_sb[:, 0:1], immask_sb[:, 0:1])
        nc.vector.tensor_copy(dummy_sb[:, 1:2], smask_sb[:, 0:1])
        nc.vector.tensor_copy(dummy_sb[:, 2:3], e_sb[:, 0:1])

        def transpose_nat(nat16, dst, dst_off):
            """8 PE transposes of one [128, D] fp16 tile into one PSUM bank,
            then a single DVE copy into dst[:, :, dst_off:dst_off+128]."""
            pt = pst.tile([128, KT, 128], F16)
            for k in range(KT):
                nc.tensor.transpose(
                    pt[:, k], nat16[:, k * 128:(k + 1) * 128], ident[:]
                )
            nc.vector.tensor_copy(dst[:, :, dst_off:dst_off + 128], pt[:])

        # Phase A: own sentences -> zero padded word rows -> transposed
        # [d, row] layout. Zeroing the word vectors up front makes every
        # masked word's align column all-zero, so its max_i is 0 and it
        # adds nothing to the j-sum - identical to the reference's mask.
        for tau in range(NTAU):
            nat = natp.tile([128, D], F16, tag="nat")
            nc.sync.dma_start(nat[:], s_own[tau * 128:(tau + 1) * 128, :])
            nat16 = nat16p.tile([128, D], F16, tag="nat16")
            nc.vector.tensor_scalar_mul(nat16[:], nat[:], smask_sb[:, tau:tau + 1])
            transpose_nat(nat16, sT[:, tau], 0)

        # Phase B: stream image groups; transpose, matmul, segment-max.
        # Software-pipelined: group g+1's load/cast/transpose work is
        # emitted before group g's matmuls so the in-order DVE stream
        # serves next-group casts before this group's reduces.
        def prep_group(g):
            if g >= 2:
                # read the last reduce output of group g-2: DVE inherits
                # that reduce's PE progress, covering this group's buffer
                # recycle dependencies without extra PE waits
                col = 7 * 128 + (g - 2) * (GROUP_ROWS // 64) + (GROUP_ROWS // 64) - 1
                nc.vector.tensor_copy(tch[:], m_all[:, col:col + 1])
            imt = imtp.tile([128, KT, GROUP_ROWS], F16, tag="imt", name=f"imt{g}")
            for n in range(NAT_PER_GROUP):
                nidx = g * NAT_PER_GROUP + n
                nat = natp.tile([128, D], F16, tag="nat", name=f"nat{nidx}")
                nc.sync.dma_start(nat[:], im_all[nidx * 128:(nidx + 1) * 128, :])
                nat16 = nat16p.tile([128, D], F16, tag="nat16", name=f"nat16{nidx}")
                # zero padded image rows; on DVE (not ACT) so the engine-
                # observed clock covers the nat16 recycle dependency.
                nc.vector.tensor_scalar_mul(
                    nat16[:], nat[:], immask_sb[:, nidx:nidx + 1]
                )
                transpose_nat(nat16, imt, n * 128)
            return imt

        imt = prep_group(0)
        for g in range(G):
            imt_next = prep_group(g + 1) if g + 1 < G else None

            # k-outer so one LDWEIGHTS serves CCH matmuls
            for tau in range(NTAU):
                pms = [psm.tile([128, 512], F32, tag="pm", name=f"pm{c}") for c in range(CCH)]
                for k in range(KT):
                    for c in range(CCH):
                        nc.tensor.matmul(
                            pms[c][:],
                            sT[:, tau, k, :],
                            imt[:, k, c * 512:(c + 1) * 512],
                            start=(k == 0),
                            stop=(k == KT - 1),
                        )
                for c in range(CCH):
                    base = tau * 128 + g * (GROUP_ROWS // 64) + c * 8
                    nc.vector.reduce_max(
                        m_all[:, base:base + 8],
                        pms[c][:].rearrange("p (i j) -> p i j", j=LI),
                        axis=mybir.AxisListType.X,
                    )
            imt = imt_next

        # Phase C: sum over words via ones-matmul (word masking already
        # applied to s itself in phase A).
        with tc.tile_pool(name="psf", bufs=1, space="PSUM") as psf:
            out_sb = outp.tile([2, TJ], F32)
            for h in range(2):
                pf = psf.tile([2, 512], F32, tag="pf")
                nc.tensor.matmul(
                    pf[:],
                    e_sb[:],
                    m_all[:, h * 512:(h + 1) * 512],
                    start=True,
                    stop=True,
                )
                nc.vector.tensor_copy(out_sb[:, h * 512:(h + 1) * 512], pf[:])
            nc.sync.dma_start(aggr_out[:], out_sb[:])

    if prune:
        _prune_redundant_waits(nc)
    return nc


def _prune_redundant_waits(nc):
    """Drop semaphore waits that are provably redundant on the final schedule.

    Walrus's per-instruction ISA structs encode very few sync waits (one for
    PE Matmult / HWDGE DMA), and Tile's wait placement leaves redundant ones:
    (a) waits on the instruction's own processor semaphore (PE matmuls
    complete in program order; a HWDGE queue executes its descriptors FIFO),
    and (b) waits whose target completion is already in the causal past of
    another wait kept on the same instruction. Both classes are dropped here
    using a conservative happens-before computed from the untouched program.

    "Processor" is the engine, except DMACopy where it is the HW queue
    (identified by its update semaphore). Ldweights can be pulled ahead of
    in-flight matmuls by the PE, so it neither extends nor inherits the
    same-proc completion chain.
    """
    insts = []
    for f in nc.m.functions:
        for bb in f.blocks:
            insts.extend(bb.instructions)

    def proc_of(i, idx):
        if i.opcode == "DMACopy":
            ups = i.sync_info.on_update
            qs = [u.ant_name for u in ups if "DMA" in u.ant_name]
            if len(qs) == 1:
                return qs[0]
            return f"__solo_{idx}"
        return f"__eng_{i.engine}"

    # completion clocks: clock[i] = {sem: min guaranteed value when i completes}
    sem_events = {}   # sem -> list of (cumval, inst_idx) in inc order
    sem_cum = {}
    clocks = [None] * len(insts)
    last_in_proc = {}

    def join(a, b):
        for k, v in b.items():
            if a.get(k, -1) < v:
                a[k] = v
        return a

    def producer_clock(sem, val):
        evs = sem_events.get(sem)
        if not evs:
            return None
        # first event reaching val
        import bisect
        pos = bisect.bisect_left(evs, (val, -1))
        if pos == len(evs):
            return None
        return clocks[evs[pos][1]]

    class _EmptySI:
        on_wait = ()
        on_update = ()

    for idx, i in enumerate(insts):
        si = i.sync_info or _EmptySI
        c = {}
        if i.opcode != "Ldweights":
            p = proc_of(i, idx)
            prev = last_in_proc.get(p)
            if prev is not None:
                join(c, clocks[prev])
            last_in_proc[p] = idx
        for w in si.on_wait:
            pc = producer_clock(w.ant_name, w.wait_value)
            if pc is not None:
                join(c, pc)
            if c.get(w.ant_name, -1) < w.wait_value:
                c[w.ant_name] = w.wait_value
        for u in si.on_update:
            sem = u.ant_name
            cum = sem_cum.get(sem, 0) + u.update_value
            sem_cum[sem] = cum
            sem_events.setdefault(sem, []).append((cum, idx))
            if c.get(sem, -1) < cum:
                c[sem] = cum
        clocks[idx] = c

    # pruning pass, walking issue order per processor:
    #   (a) waits on the instruction's own processor semaphore (in-order
    #       completion within a processor),
    #   (b) waits transitively covered by another kept wait's causal past,
    #   (c) waits at-or-below what an earlier instruction on the same
    #       issue processor already waited for (semaphores are monotone).
    PRUNABLE = {
        "Matmult", "Ldweights", "DMACopy", "Activation", "TensorCopy",
        "TensorReduce", "TensorScalarPtr", "TensorTensor", "Memset",
        "Drain",
    }
    stripped = 0
    proc_hist = {}   # proc -> recent [(idx, inst, proc_sem_cum_after)]
    proc_sem = {}    # proc -> its completion semaphore name
    upd_cum = {}     # sem -> cumulative update value (pruning pass copy)
    # issue proc -> clock of everything provably completed before the
    # proc's current issue point (prior waits' targets AND their causal
    # pasts — a satisfied wait implies its producer's whole past, and
    # semaphores are monotone)
    observed = {}

    for idx, i in enumerate(insts):
        si = i.sync_info
        if si is None:
            continue
        p = proc_of(i, idx)
        obs = observed.setdefault(p, {})
        waits = list(si.on_wait)
        a_dropped = []
        if i.opcode in PRUNABLE and waits:
            eng = str(i.engine).split(".")[-1]
            kept = []
            for w in waits:
                sem_eng = w.ant_name.rsplit("_", 1)[0]
                # rule (a): same-engine completion is in program order, so a
                # wait on the engine's own semaphore is vacuous. NOT applied
                # to DMA self-queue waits: a queue's sem increments are only
                # ordered if the previous transfer provably completed, which
                # is rule (b)'s job. Dropped waits still hold at execution
                # time (FIFO engines execute in order), so they remain
                # usable as cover and observation.
                if i.opcode != "DMACopy" and sem_eng == eng:
                    a_dropped.append(w)
                    continue
                if obs.get(w.ant_name, -1) >= w.wait_value:
                    continue           # rule (c): already observed
                kept.append(w)
            # rule (b): transitive cover by other kept or (a)-dropped waits
            changed = True
            while changed and len(kept) > 1:
                changed = False
                for w in list(kept):
                    cover = {}
                    for x in kept + a_dropped:
                        if x is w:
                            continue
                        pc = producer_clock(x.ant_name, x.wait_value)
                        if pc is not None:
                            join(cover, pc)
                    if cover.get(w.ant_name, -1) >= w.wait_value:
                        kept.remove(w)
                        changed = True
            # fallback: migrate excess waits to an earlier same-proc
            # instruction with a free wait slot. Moving a wait earlier on
            # the issuing processor only strengthens ordering; it cannot
            # deadlock as long as the wait's producer does not causally
            # depend on the target instruction or anything after it on
            # this proc (checked via the producer's clock).
            while len(kept) > 1:
                w = kept[-1]
                pcw = producer_clock(w.ant_name, w.wait_value) or {}
                placed = False
                for t_idx, t_inst, t_cum in reversed(proc_hist.get(p, [])):
                    if t_inst.sync_info is None:
                        continue
                    psem = proc_sem.get(p)
                    if psem is not None and pcw.get(psem, -1) >= t_cum:
                        break  # producer needs this inst or later: stop
                    tw = list(t_inst.sync_info.on_wait)
                    if len(tw) == 0:
                        t_inst.sync_info.on_wait = [w]
                    elif len(tw) == 1 and tw[0].ant_name == w.ant_name:
                        if tw[0].wait_value < w.wait_value:
                            t_inst.sync_info.on_wait = [w]
                    else:
                        continue
                    kept.remove(w)
                    placed = True
                    break
                if not placed:
                    break
            if len(kept) != len(waits):
                si.on_wait = kept
                stripped += 1
            waits = kept
        for w in list(waits) + a_dropped:
            if obs.get(w.ant_name, -1) < w.wait_value:
                obs[w.ant_name] = w.wait_value
            pc = producer_clock(w.ant_name, w.wait_value)
            if pc is not None:
                join(obs, pc)
        cum = None
        for u in (si.on_update or ()):
            sem_eng_u = u.ant_name.rsplit("_", 1)[0]
            if sem_eng_u == str(i.engine).split(".")[-1] or "DMA" in u.ant_name:
                proc_sem[p] = u.ant_name
                cum = upd_cum.get(u.ant_name, 0) + u.update_value
                upd_cum[u.ant_name] = cum
        proc_hist.setdefault(p, []).append(
            (idx, i, cum if cum is not None else upd_cum.get(proc_sem.get(p, ""), 0))
        )
        if len(proc_hist[p]) > 64:
            proc_hist[p] = proc_hist[p][-64:]
    return stripped


_NC_CACHE = None


def _get_nc():
    global _NC_CACHE
    if _NC_CACHE is None:
        _NC_CACHE = _build_nc()
    return _NC_CACHE


def _prepare_in_maps(im_set, s_seq, im_len, s_len):
    im_set = np.asarray(im_set, dtype=np.float32)
    s_seq = np.asarray(s_seq, dtype=np.float32)
    im_l = np.asarray(im_len).astype(np.int64) - 1
    s_l = np.asarray(s_len).astype(np.int64) - 3

    im = np.ascontiguousarray(im_set[:, 1:, :]).reshape(BI, D).astype(np.float16)
    immask_full = (np.arange(LI)[None, :] < im_l[:, None]).astype(np.float32)
    immask_dev = np.ascontiguousarray(
        immask_full.reshape(BI // 128, 128).T
    )
    smask_full = (np.arange(LS)[None, :] < s_l[:, None]).astype(np.float32)
    smask_flat = smask_full.reshape(B * LS)

    ones_e = np.zeros((128, 2), np.float16)
    ones_e[:64, 0] = 1.0
    ones_e[64:, 1] = 1.0
    ident = np.eye(128, dtype=np.float16)

    in_maps = []
    for c in range(NCORES):
        s_own = np.ascontiguousarray(
            s_seq[c * T_PER_CORE:(c + 1) * T_PER_CORE, 1:1 + LS, :]
        ).reshape(TJ, D).astype(np.float16)
        smask_own = np.ascontiguousarray(
            smask_flat[c * TJ:(c + 1) * TJ].reshape(NTAU, 128).T
        )
        in_maps.append(
            {
                "s_own": s_own,
                "im_all": im,
                "immask": immask_dev,
                "smask": smask_own,
                "ones_e": ones_e,
                "ident_in": ident,
            }
        )
    return in_maps


def _loss_from_cores(core_outs):
    aggr = np.zeros((B, B), np.float64)
    for c in range(NCORES):
        o = np.asarray(core_outs[c], dtype=np.float64).reshape(2, NTAU, 128)
        for tau in range(NTAU):
            for h in range(2):
                aggr[:, c * T_PER_CORE + 2 * tau + h] = o[h, tau, :]
    diag = np.diag(aggr)
    cost_s = MARGIN + aggr - diag[:, None]
    cost_im = MARGIN + aggr - diag[None, :]
    np.fill_diagonal(cost_s, 0.0)
    np.fill_diagonal(cost_im, 0.0)
    cost_s = np.maximum(cost_s, 0.0)
    cost_im = np.maximum(cost_im, 0.0)
    loss = cost_s.max(axis=1).sum() + cost_im.max(axis=0).sum()
    return np.array(loss, dtype=np.float32)


def _run(im_set, s_seq, im_len, s_len, **spmd_kwargs):
    nc = _get_nc()
    in_maps = _prepare_in_maps(im_set, s_seq, im_len, s_len)
    res = run_bass_kernel_spmd(
        nc, in_maps, core_ids=list(range(NCORES)), **spmd_kwargs
    )
    loss = _loss_from_cores([r["aggr_out"] for r in res.results])
    return loss, res


def kernel(im_set, s_seq, im_len, s_len):
    loss, _ = _run(im_set, s_seq, im_len, s_len)
    return loss


def _install_ntff_hook_shim():
    """This image's antenv lacks axon_hooks; recreate it from trn_boot's
    ctypes path so run_bass_kernel_spmd(trace=True) can capture NTFFs."""
    import sys
    import types

    if "antenv.axon_hooks" in sys.modules:
        return
    from trn_agent_boot.trn_boot import _ntff_profile_via_ctypes

    hook = _ntff_profile_via_ctypes("/opt/axon/libaxon_pjrt.so")
    mod = types.ModuleType("antenv.axon_hooks")
    mod._hook = hook
    mod.get_axon_ntff_profile_hook = lambda: mod._hook
    mod.set_axon_ntff_profile_hook = lambda h: setattr(mod, "_hook", h)
    sys.modules["antenv.axon_hooks"] = mod
    import antenv

    antenv.axon_hooks = mod


def kernel_traced(im_set, s_seq, im_len, s_len, **kwargs):
    """Returns (loss, BassKernelResults-with-exec_time_ns)."""
    _install_ntff_hook_shim()
    loss, res = _run(im_set, s_seq, im_len, s_len, trace=True, **kwargs)
    return loss, res
